# revision 2
# baseline (speedup 1.0000x reference)
import numpy as np

# Problem constants (hardcoded from spec)
N_AGENTS, N_ENEMIES, N_ACTIONS = 8, 8, 14
ALLY_F, ENEMY_F = 8, 6
EMBED, RNN, HYPEMB, HYPHID = 32, 64, 64, 64
STATE_DIM = N_AGENTS * ALLY_F + N_ENEMIES * ENEMY_F + N_AGENTS * N_ACTIONS  # 224
NCORES = 8

_PARAM_NAMES = (
    "hw1_w", "hw1_b",
    "en_h1w", "en_h1b", "en_h2w", "en_h2b", "en_bias",
    "al_h1w", "al_h1b", "al_h2w", "al_h2b", "al_bias",
    "act_w", "act_b", "hb1_w", "hb1_b", "hw2_w", "hw2_b", "hb2_w", "hb2_b",
)

# ---------------------------------------------------------------------------
# numpy reference-equivalent forward (always available, device-independent).
# The per-sample hypernet weight einsum is rewritten as one dense matmul:
#   out[b,o] = sum_i x[b,i] * (h[b] @ h2w)[i*HYPEMB+o]
#            = sum_{j,i} h[b,j] x[b,i] h2w[j, i*HYPEMB+o]
#            = (outer(h,x).flat) @ M   with M[(j,i),o] = h2w[j, i*HYPEMB+o]
# plus the h2b contribution x @ B with B[i,o] = h2b[i*HYPEMB+o].
# ---------------------------------------------------------------------------


def _hyper_dense_np(x, h1w, h1b, Mw, Bw, bias):
    h = np.maximum(x @ h1w + h1b, 0.0)
    Z = (h[:, :, None] * x[:, None, :]).reshape(x.shape[0], -1)
    return Z @ Mw + x @ Bw + bias


def _forward_np(q, s, hs, p):
    n = q.shape[0]
    ally = s[:, : N_AGENTS * ALLY_F].reshape(-1, ALLY_F)
    enemy = s[:, N_AGENTS * ALLY_F : N_AGENTS * ALLY_F + N_ENEMIES * ENEMY_F].reshape(-1, ENEMY_F)
    actions = s[:, N_AGENTS * ALLY_F + N_ENEMIES * ENEMY_F :].reshape(n, N_AGENTS, N_ACTIONS)

    al_Mw = p["al_h2w"].reshape(HYPHID, ALLY_F, HYPEMB).reshape(HYPHID * ALLY_F, HYPEMB)
    al_Bw = p["al_h2b"].reshape(ALLY_F, HYPEMB)
    en_Mw = p["en_h2w"].reshape(HYPHID, ENEMY_F, HYPEMB).reshape(HYPHID * ENEMY_F, HYPEMB)
    en_Bw = p["en_h2b"].reshape(ENEMY_F, HYPEMB)

    ea = _hyper_dense_np(ally, p["al_h1w"], p["al_h1b"], al_Mw, al_Bw, p["al_bias"])
    embed_ally = ea.reshape(n, N_AGENTS, HYPEMB).mean(axis=1)
    ee = _hyper_dense_np(enemy, p["en_h1w"], p["en_h1b"], en_Mw, en_Bw, p["en_bias"])
    embed_enemy = ee.reshape(n, N_ENEMIES, HYPEMB).mean(axis=1)
    embed_action = (actions.reshape(n * N_AGENTS, N_ACTIONS) @ p["act_w"] + p["act_b"]) \
        .reshape(n, N_AGENTS, HYPEMB).mean(axis=1)
    se = np.maximum(embed_ally + embed_enemy + embed_action, 0.0)

    logits = (hs.reshape(n * N_AGENTS, RNN) @ p["hw1_w"] + p["hw1_b"]).reshape(n, N_AGENTS, EMBED)
    logits -= logits.max(axis=1, keepdims=True)
    ex = np.exp(logits)
    w1 = ex / ex.sum(axis=1, keepdims=True)

    b1 = se @ p["hb1_w"] + p["hb1_b"]
    w2 = np.abs(se @ p["hw2_w"] + p["hw2_b"])
    b2 = se @ p["hb2_w"] + p["hb2_b"]

    pre = np.einsum("na,nae->ne", q, w1) + b1
    hidden = np.where(pre > 0, pre, np.expm1(np.minimum(pre, 0.0)))
    y = np.sum(hidden * w2, axis=1, keepdims=True) + b2
    return y.astype(np.float32)


# ---------------------------------------------------------------------------
# trn2 path: same math via jax pmap across the 8 NeuronCores. Attempted under
# a watchdog; any failure or hang falls back to the numpy result.
# ---------------------------------------------------------------------------

_jax_state = {"fn": None, "failed": False}


def _try_jax_forward(q, s, hs, p, timeout_s=240.0):
    import threading

    result = {}

    def _run():
        try:
            import jax
            import jax.numpy as jnp

            if _jax_state["fn"] is None:
                def _hyper(x, h1w, h1b, Mw, Bw, bias):
                    h = jax.nn.relu(x @ h1w + h1b)
                    Z = (h[:, :, None] * x[:, None, :]).reshape(x.shape[0], -1)
                    return Z @ Mw + x @ Bw + bias

                def _fwd(q, s, hs, params):
                    (hw1_w, hw1_b, en_h1w, en_h1b, en_Mw, en_Bw, en_bias,
                     al_h1w, al_h1b, al_Mw, al_Bw, al_bias,
                     act_w, act_b, hb1_w, hb1_b, hw2_w, hw2_b, hb2_w, hb2_b) = params
                    n = q.shape[0]
                    ally = s[:, : N_AGENTS * ALLY_F].reshape(-1, ALLY_F)
                    enemy = s[:, N_AGENTS * ALLY_F : N_AGENTS * ALLY_F + N_ENEMIES * ENEMY_F].reshape(-1, ENEMY_F)
                    actions = s[:, N_AGENTS * ALLY_F + N_ENEMIES * ENEMY_F :].reshape(n, N_AGENTS, N_ACTIONS)
                    ea = _hyper(ally, al_h1w, al_h1b, al_Mw, al_Bw, al_bias)
                    embed_ally = ea.reshape(n, N_AGENTS, HYPEMB).mean(axis=1)
                    ee = _hyper(enemy, en_h1w, en_h1b, en_Mw, en_Bw, en_bias)
                    embed_enemy = ee.reshape(n, N_ENEMIES, HYPEMB).mean(axis=1)
                    embed_action = (actions @ act_w + act_b).mean(axis=1)
                    se = jax.nn.relu(embed_ally + embed_enemy + embed_action)
                    w1 = jax.nn.softmax(hs @ hw1_w + hw1_b, axis=1)
                    b1 = se @ hb1_w + hb1_b
                    w2 = jnp.abs(se @ hw2_w + hw2_b)
                    b2 = se @ hb2_w + hb2_b
                    hidden = jax.nn.elu(jnp.einsum("na,nae->ne", q, w1) + b1)
                    return jnp.sum(hidden * w2, axis=1, keepdims=True) + b2

                import jax as _j
                _jax_state["fn"] = _j.pmap(_fwd, in_axes=(0, 0, 0, None),
                                           devices=_j.devices()[:NCORES])

            BTl = q.shape[0] // NCORES
            qs = q.reshape(NCORES, BTl, N_AGENTS)
            ss = s.reshape(NCORES, BTl, STATE_DIM)
            hss = hs.reshape(NCORES, BTl, N_AGENTS, RNN)

            al_Mw = p["al_h2w"].reshape(HYPHID, ALLY_F, HYPEMB).reshape(HYPHID * ALLY_F, HYPEMB)
            al_Bw = p["al_h2b"].reshape(ALLY_F, HYPEMB)
            en_Mw = p["en_h2w"].reshape(HYPHID, ENEMY_F, HYPEMB).reshape(HYPHID * ENEMY_F, HYPEMB)
            en_Bw = p["en_h2b"].reshape(ENEMY_F, HYPEMB)
            params = (
                p["hw1_w"], p["hw1_b"],
                p["en_h1w"], p["en_h1b"], en_Mw, en_Bw, p["en_bias"],
                p["al_h1w"], p["al_h1b"], al_Mw, al_Bw, p["al_bias"],
                p["act_w"], p["act_b"], p["hb1_w"], p["hb1_b"],
                p["hw2_w"], p["hw2_b"], p["hb2_w"], p["hb2_b"],
            )
            y = _jax_state["fn"](qs, ss, hss, params)
            result["y"] = np.asarray(y).reshape(-1, 1).astype(np.float32)
        except Exception as e:  # noqa: BLE001
            result["err"] = e

    th = threading.Thread(target=_run, daemon=True)
    th.start()
    th.join(timeout_s)
    if th.is_alive() or "err" in result or "y" not in result:
        _jax_state["failed"] = True
        return None
    return result["y"]


def kernel(**inputs):
    qvals = np.ascontiguousarray(inputs["qvals"], np.float32)
    b, t, _ = qvals.shape
    BT = b * t
    q = qvals.reshape(BT, N_AGENTS)
    s = np.ascontiguousarray(inputs["states"], np.float32).reshape(BT, STATE_DIM)
    hs = np.ascontiguousarray(inputs["hidden_states"], np.float32).reshape(BT, N_AGENTS, RNN)
    p = {k: np.ascontiguousarray(inputs[k], np.float32) for k in _PARAM_NAMES}

    y = None
    if not _jax_state["failed"]:
        y = _try_jax_forward(q, s, hs, p)
    if y is None:
        y = _forward_np(q, s, hs, p)
    return y.reshape(b, t, 1)
